# revision 19
# baseline (speedup 1.0000x reference)
"""Causal ReLU-attention (no softmax) fused kernel for TRN2, 8 NeuronCores.

Reference computation (B=2, T=2048, C=1024, H=16, D=64):
    qkv = x @ W.T + b ; q,k,v split; per (b,h): y = relu(tril(q k^T / sqrt(D))) @ v

Sharding: core c handles batch b = c//4 and heads 4*(c%4) .. 4*(c%4)+3.
Each core is fully independent (no collectives).

All-bf16 datapath (inputs cast host-side; PSUM accumulation stays fp32;
measured end-to-end rel err ~5.9e-3 vs the 2e-2 gate).

v8 architecture.  The kernel is bound by three coupled limits: PE
streaming (projection 41us at 1 col/cyc full width; tiled ST/AV pairs
~2 cols/cyc), PSUM->SBUF score evacuation on ACT+DVE (~543ns/block
combined -- 2.5x an ST block's PE cost; DMA has NO PSUM route so these
two engines are the only drain), and a ~200-320ns PE drain per
tiling-mode switch.  A pure two-phase schedule pins phase 2 at the
evacuation floor (80 blocks x 543ns = 43us); quarter-granular phase 1
starves the DMA lead-in (8 cores share HBM; quarter chunks need ~190GB/s
vs ~130GB/s for half chunks).  So:

  half-0   projection for t 0:1024 exactly as the proven two-phase v1
           (big psum tiles, DMA-bandwidth-friendly chunking, no
           insertions -- the lead-in is DMA-gated anyway).
  quarters projection for t 1024:1536 and 1536:2048 at quarter
  2,3      granularity (4 psum banks), with the STs of qc0+qc1 (24
           blocks) inserted between quarter-2's chunks and qc2's 24
           between quarter-3's -- the projection matmuls hide their
           evacuation entirely.
  final    qc3's 32 STs in clumps of 3 interleaved with ALL 80 AV
           blocks (AVs produce no evacuation load and cover the tail
           STs' evacuation); AVs lead so the segment entry never idles.
           Output flushes per (head-pair, qc) as each AV accumulation
           completes; the last query chunk flushes in four pieces with
           the last two DMAs split across the Sync and Pool queues.

Output per core: yT [256, 2048] bf16; host upcasts/transposes into
y[b, :, 256g:256g+256].
"""

import numpy as np

N_EMBD = 1024
N_HEAD = 16
HEAD_DIM = 64
B, T, C = 2, 2048, N_EMBD
NCORES = 8
P = 128
KC = C // P  # 8 contraction chunks
NQC = T // 512  # 4 query chunks

_NC_CACHE = {}


def _build_bass():
    import concourse.bass as bass
    from concourse import bacc, mybir
    from concourse.tile import TileContext

    f32 = mybir.dt.float32
    bf16 = mybir.dt.bfloat16

    nc = bacc.Bacc(None, target_bir_lowering=False)
    xt = nc.declare_dram_parameter("xt", [C, T], bf16, isOutput=False)
    wt = nc.declare_dram_parameter("wt", [C, 768], bf16, isOutput=False)
    bcol = nc.declare_dram_parameter("bcol", [512], f32, isOutput=False)
    bv = nc.declare_dram_parameter("bv", [256], f32, isOutput=False)
    out = nc.declare_dram_parameter("out", [256, T], bf16, isOutput=True)

    xt_r = xt[:, :].rearrange("(c p) t -> c p t", p=P)
    wt_r = wt[:, :].rearrange("(c p) o -> c p o", p=P)
    bv_ap = bv[:]
    HT = T // 2

    with TileContext(nc) as tc:
        with (
            tc.tile_pool(name="const", bufs=1) as const_pool,
            tc.tile_pool(name="qkv", bufs=1) as qkv_pool,
            tc.tile_pool(name="stsb", bufs=52) as stsb_pool,
            tc.tile_pool(name="ysb", bufs=6) as ysb_pool,
        ):
            # ACT act-table pre-warm: first ACTIVATE triggers a ~1.3us
            # ACT_TABLE_LOAD; run it here so it overlaps the DMA lead-in.
            warm = const_pool.tile([1, 8], f32, name="warm")
            nc.vector.memset(warm, 0.0)
            nc.scalar.activation(
                out=warm, in_=warm, func=mybir.ActivationFunctionType.Relu
            )

            xt_h0 = const_pool.tile([P, KC, HT], bf16, name="xt_h0")
            xt_q = [
                const_pool.tile([P, KC, 512], bf16, name=f"xt_q{w}")
                for w in range(2, 4)
            ]
            wt_sb = const_pool.tile([P, KC, 768], bf16)
            bqk_sb = const_pool.tile([P, 4], f32)
            bv_rep = const_pool.tile([P, 256], f32)
            bv_bcast = bass.AP(
                tensor=bv_ap.tensor, offset=bv_ap.offset, ap=[[0, P], [1, 256]]
            )
            # DMA order == consumption order.  chunk 0 split in halves so
            # the first q/k matmuls start ~1us earlier; biases follow the
            # first sub-chunks (needed only by the ~18us evacs).  Quarter-3
            # xt rides the (slow-issue, early-idle) GpSimd queue to unclog
            # Sync's issue stream.
            nc.sync.dma_start(out=wt_sb[:, 0, 0:512], in_=wt_r[0][:, 0:512])
            nc.sync.dma_start(out=xt_h0[:, 0, 0:512], in_=xt_r[0][:, 0:512])
            nc.sync.dma_start(out=bqk_sb, in_=bcol[:].rearrange("(m p) -> p m", p=P))
            nc.sync.dma_start(out=bv_rep, in_=bv_bcast)
            nc.sync.dma_start(out=wt_sb[:, 0, 512:768], in_=wt_r[0][:, 512:768])
            nc.sync.dma_start(out=xt_h0[:, 0, 512:HT], in_=xt_r[0][:, 512:HT])
            for c in range(1, KC):
                nc.sync.dma_start(out=wt_sb[:, c, :], in_=wt_r[c])
                nc.sync.dma_start(out=xt_h0[:, c, :], in_=xt_r[c][:, 0:HT])
            for c in range(KC):
                nc.sync.dma_start(out=xt_q[0][:, c, :], in_=xt_r[c][:, 1024:1536])
            for c in range(KC):
                nc.gpsimd.dma_start(out=xt_q[1][:, c, :], in_=xt_r[c][:, 1536:2048])

            # PE p-state warm-up bridge while the first DMA chunks land.
            pe_warm = const_pool.tile([P, 512], bf16, name="pe_warm")
            nc.vector.memset(pe_warm, 0.0)
            with tc.tile_pool(name="psw", bufs=1, space="PSUM") as psw_pool:
                psw = psw_pool.tile([P, 512], f32, name="psw")
                for _ in range(2):
                    nc.tensor.matmul(
                        psw, pe_warm[:, 0:128], pe_warm, start=True, stop=True
                    )

            q_sb = qkv_pool.tile([P, 2, T], bf16)
            k_sb = qkv_pool.tile([P, 2, T], bf16)
            v_sb = qkv_pool.tile([P, T // P, 256], bf16)

            bv_in = bass.AP(
                tensor=bv_rep.tensor,
                offset=bv_rep.offset,
                ap=[bv_rep.ap[0], [0, 2], [1, 256]],
            )

            # ---------- phase-2 building blocks ----------
            evac_ctr = 0
            out_ctr = 0
            stsb_map = {}
            yps = [None, None]

            def st_block(pst_pool, hp, qc, kb):
                nonlocal evac_ctr
                d = kb - 4 * qc
                c0 = P * d if d > 0 else 0
                stps = pst_pool.tile([P, 2, 512], f32, tag="stps", name="stps")
                stsb = stsb_pool.tile([P, 2, 512], bf16, tag="stsb", name="stsb")
                stsb_map[(hp, qc, kb)] = stsb
                for hh in range(2):
                    off = hh * 64
                    nc.tensor.matmul(
                        stps[:, hh, c0:512],
                        k_sb[off : off + 64, hp, kb * P : (kb + 1) * P],
                        q_sb[off : off + 64, hp, qc * 512 + c0 : (qc + 1) * 512],
                        start=True,
                        stop=True,
                    )
                # ReLU evac of the whole block on ONE engine (alternating):
                # fewer, bigger ops beat split ops on fixed overheads.
                if evac_ctr % 2 == 0:
                    nc.scalar.activation(
                        out=stsb[:, :, c0:512],
                        in_=stps[:, :, c0:512],
                        func=mybir.ActivationFunctionType.Relu,
                    )
                else:
                    nc.vector.tensor_scalar_max(
                        stsb[:, :, c0:512], stps[:, :, c0:512], 0.0
                    )
                evac_ctr += 1
                if d >= 0:
                    # triangular mask on the diag 128-col tile, both heads
                    # in one strided op on GpSimd (SBUF in-place)
                    base = stsb[:, 0, P * d : P * d + P]
                    tri = bass.AP(
                        tensor=base.tensor,
                        offset=base.offset,
                        ap=[base.ap[0], [512, 2], [1, P]],
                    )
                    nc.gpsimd.affine_select(
                        out=tri,
                        in_=tri,
                        pattern=[[0, 2], [1, P]],
                        compare_op=mybir.AluOpType.is_ge,
                        fill=0.0,
                        base=0,
                        channel_multiplier=-1,
                    )

            def emit_out(hp, qc, lo, hi):
                nonlocal out_ctr
                ysb = ysb_pool.tile([P, 512], bf16, tag="ysb", name="ysb")
                r = out_ctr % 2
                out_ctr += 1
                if r == 0:
                    nc.scalar.copy(ysb[:, lo:hi], yps[hp][:, lo:hi])
                else:
                    nc.vector.tensor_copy(ysb[:, lo:hi], yps[hp][:, lo:hi])
                # Non-final output DMAs issue from the Pool queue; the final
                # flush's last two pieces split across Sync and Pool so their
                # issues run in parallel instead of serializing ~600ns each
                # into the kernel tail.
                eng = nc.sync if (qc == NQC - 1 and lo == 384) else nc.gpsimd
                eng.dma_start(
                    out=out[hp * P : (hp + 1) * P, qc * 512 + lo : qc * 512 + hi],
                    in_=ysb[:, lo:hi],
                )

            def av_block(py_pool, hp, qc, kb):
                nblocks = 4 * (qc + 1)
                first, last = kb == 0, kb == nblocks - 1
                d = kb - 4 * qc
                c0 = P * d if d > 0 else 0
                stsb = stsb_map.pop((hp, qc, kb))
                if first:
                    yps[hp] = py_pool.tile(
                        [P, 512], f32, tag=f"yps{hp}", name=f"yps{hp}"
                    )
                for hh in range(2):
                    h = 2 * hp + hh
                    nc.tensor.matmul(
                        yps[hp][hh * 64 : (hh + 1) * 64, c0:512],
                        v_sb[:, kb, h * 64 : (h + 1) * 64],
                        stsb[:, hh, c0:512],
                        start=first,
                        stop=last,
                    )
                # output evac/DMA; the final query chunk flushes in four
                # pieces as its diagonal AVs finalize columns.
                if qc == NQC - 1:
                    if kb == 4 * qc + 1:
                        emit_out(hp, qc, 0, 128)
                    elif kb == 4 * qc + 2:
                        emit_out(hp, qc, 128, 384)
                    elif last:
                        emit_out(hp, qc, 384, 480)
                        emit_out(hp, qc, 480, 512)
                elif last:
                    emit_out(hp, qc, 0, 512)

            # ---------- half-0: projection for t 0:1024 (v1 structure) ----------
            # q: 2 x [128,1024] psum (4 banks), ACT evac (bias via activation).
            # k: 4 x [128,512] psum (4 banks), DVE/ACT evacs; v reuses the k
            # banks piecewise.
            with tc.tile_pool(name="psum1", bufs=1, space="PSUM") as psum1:
                pq = [
                    psum1.tile([P, HT], f32, tag=f"psA{m}", name=f"pq{m}")
                    for m in range(2)
                ]
                pk = [
                    psum1.tile([P, 512], f32, tag=f"psB{i}", name=f"pk{i}")
                    for i in range(4)
                ]
                for c in range(KC):
                    for m in range(2):
                        for n in range(2):
                            nc.tensor.matmul(
                                pq[m][:, n * 512 : (n + 1) * 512],
                                wt_sb[:, c, m * P : (m + 1) * P],
                                xt_h0[:, c, n * 512 : (n + 1) * 512],
                                start=(c == 0),
                                stop=(c == KC - 1),
                            )
                    for m in range(2):
                        for n in range(2):
                            nc.tensor.matmul(
                                pk[2 * m + n],
                                wt_sb[:, c, 256 + m * P : 256 + (m + 1) * P],
                                xt_h0[:, c, n * 512 : (n + 1) * 512],
                                start=(c == 0),
                                stop=(c == KC - 1),
                            )
                for i in range(4):
                    m, n = i // 2, i % 2
                    if i % 2 == 0:
                        nc.vector.tensor_scalar_add(
                            k_sb[:, m, n * 512 : (n + 1) * 512],
                            pk[i],
                            bqk_sb[:, 2 + m : 3 + m],
                        )
                    else:
                        nc.scalar.activation(
                            out=k_sb[:, m, n * 512 : (n + 1) * 512],
                            in_=pk[i],
                            func=mybir.ActivationFunctionType.Identity,
                            bias=bqk_sb[:, 2 + m : 3 + m],
                            scale=1.0,
                        )
                for m in range(2):
                    nc.scalar.activation(
                        out=q_sb[:, m, 0:HT],
                        in_=pq[m],
                        func=mybir.ActivationFunctionType.Identity,
                        bias=bqk_sb[:, m : m + 1],
                        scale=1.0,
                    )
                # v: 4 pieces of 2 t-blocks, each in a freed k bank.
                bv_in2 = bass.AP(
                    tensor=bv_rep.tensor,
                    offset=bv_rep.offset,
                    ap=[bv_rep.ap[0], [0, 2], [1, 256]],
                )
                for piece in range(4):
                    pv = psum1.tile(
                        [P, 512], f32, tag=f"psB{3 - piece}", name=f"pv{piece}"
                    )
                    for tl in range(2):
                        tbl = piece * 2 + tl
                        for c in range(KC):
                            nc.tensor.matmul(
                                pv[:, tl * 256 : (tl + 1) * 256],
                                xt_h0[:, c, tbl * P : (tbl + 1) * P],
                                wt_sb[:, c, 512:768],
                                start=(c == 0),
                                stop=(c == KC - 1),
                            )
                    v_dst = v_sb[:, piece * 2 : (piece + 1) * 2, :]
                    nc.vector.scalar_tensor_tensor(
                        out=v_dst,
                        in0=pv.rearrange("p (a b) -> p a b", a=2),
                        scalar=0.0,
                        in1=bv_in2,
                        op0=mybir.AluOpType.add,
                        op1=mybir.AluOpType.add,
                    )

            # ---------- quarters 2,3 + inserted STs ----------
            # quarter 2 hosts the STs of qc0+qc1 (24 blocks, inputs from
            # half-0); quarter 3 hosts qc2's 24.  Clumps of 2 match pstq
            # depth; the next clump is a full projection chunk away so its
            # evacuation is covered.
            def st_backlog(w):
                if w == 2:
                    return [
                        (hp, qc, kb)
                        for qc in (0, 1)
                        for kb in range(4 * qc + 4)
                        for hp in range(2)
                    ]
                return [(hp, 2, kb) for kb in range(12) for hp in range(2)]

            with (
                tc.tile_pool(name="pq2", bufs=1, space="PSUM") as pq_pool,
                tc.tile_pool(name="pkv", bufs=1, space="PSUM") as pkv_pool,
                tc.tile_pool(name="pstq", bufs=2, space="PSUM") as pstq_pool,
            ):
                for wi, w in enumerate((2, 3)):
                    backlog = st_backlog(w)
                    bi = 0

                    def insert(n):
                        nonlocal bi
                        for _ in range(n):
                            if bi < len(backlog):
                                hp, qc, kb = backlog[bi]
                                st_block(pstq_pool, hp, qc, kb)
                                bi += 1

                    pqt = pq_pool.tile([P, 2, 512], f32, tag="pq", name="pq")
                    pkt = pkv_pool.tile([P, 2, 512], f32, tag="pkv", name="pk")
                    for c in range(KC):
                        for m in range(2):
                            nc.tensor.matmul(
                                pqt[:, m, :],
                                wt_sb[:, c, m * P : (m + 1) * P],
                                xt_q[wi][:, c, :],
                                start=(c == 0),
                                stop=(c == KC - 1),
                            )
                        for m in range(2):
                            nc.tensor.matmul(
                                pkt[:, m, :],
                                wt_sb[:, c, 256 + m * P : 256 + (m + 1) * P],
                                xt_q[wi][:, c, :],
                                start=(c == 0),
                                stop=(c == KC - 1),
                            )
                        if c >= 1:
                            insert(2)
                    t0c = 512 * w
                    for m in range(2):
                        nc.scalar.activation(
                            out=q_sb[:, m, t0c : t0c + 512],
                            in_=pqt[:, m, :],
                            func=mybir.ActivationFunctionType.Identity,
                            bias=bqk_sb[:, m : m + 1],
                            scale=1.0,
                        )
                    nc.vector.tensor_scalar_add(
                        k_sb[:, 0, t0c : t0c + 512], pkt[:, 0, :], bqk_sb[:, 2:3]
                    )
                    nc.scalar.activation(
                        out=k_sb[:, 1, t0c : t0c + 512],
                        in_=pkt[:, 1, :],
                        func=mybir.ActivationFunctionType.Identity,
                        bias=bqk_sb[:, 3:4],
                        scale=1.0,
                    )
                    # v: two groups of 2 t-blocks, each reusing the pkv slot
                    # (same [P,2,512] shape as pk; only cols 0:256 used).
                    for tb in range(2):
                        pv = pkv_pool.tile([P, 2, 512], f32, tag="pkv", name="pv")
                        for tl in range(2):
                            tbl = tb * 2 + tl
                            for c in range(KC):
                                nc.tensor.matmul(
                                    pv[:, tl, 0:256],
                                    xt_q[wi][:, c, tbl * P : (tbl + 1) * P],
                                    wt_sb[:, c, 512:768],
                                    start=(c == 0),
                                    stop=(c == KC - 1),
                                )
                            insert(2)
                        nc.vector.scalar_tensor_tensor(
                            out=v_sb[:, w * 4 + tb * 2 : w * 4 + (tb + 1) * 2, :],
                            in0=pv[:, :, 0:256],
                            scalar=0.0,
                            in1=bv_in,
                            op0=mybir.AluOpType.add,
                            op1=mybir.AluOpType.add,
                        )
                        insert(2)
                    insert(len(backlog))  # any remainder at quarter boundary

            # ---------- final segment: qc3 STs + all AVs ----------
            with (
                tc.tile_pool(name="pstf", bufs=3, space="PSUM") as pstf_pool,
                tc.tile_pool(name="py", bufs=1, space="PSUM") as py_pool,
            ):
                final_sts = [(hp, 3, kb) for kb in range(16) for hp in range(2)]
                avs = [
                    (hp, qc, kb)
                    for qc in range(NQC)
                    for kb in range(4 * (qc + 1))
                    for hp in range(2)
                ]
                si = ai = 0
                st_done = set(stsb_map.keys())

                # AVs lead each cycle: the segment opens with qc0's AVs
                # (inputs quarters old) so the PE never idles at the
                # quarter->final transition and HAM stays warm.
                while si < len(final_sts) or ai < len(avs):
                    navs = 7 if si < len(final_sts) else len(avs)
                    done = 0
                    while ai < len(avs) and done < navs:
                        if avs[ai] not in st_done:
                            break
                        hp, qc, kb = avs[ai]
                        av_block(py_pool, hp, qc, kb)
                        ai += 1
                        done += 1
                    for _ in range(3):
                        if si < len(final_sts):
                            hp, qc, kb = final_sts[si]
                            st_block(pstf_pool, hp, qc, kb)
                            st_done.add((hp, qc, kb))
                            si += 1

    nc.compile()
    return nc


def _get_nc():
    if "nc" not in _NC_CACHE:
        _NC_CACHE["nc"] = _build_bass()
    return _NC_CACHE["nc"]


def make_in_maps(x: np.ndarray, W: np.ndarray, b: np.ndarray):
    import ml_dtypes

    bf = ml_dtypes.bfloat16
    scale = np.float32(1.0 / np.sqrt(HEAD_DIM))
    xts = [np.ascontiguousarray(x[bb].T.astype(bf)) for bb in range(B)]
    in_maps = []
    for core in range(NCORES):
        bb, g = core // 4, core % 4
        o0 = g * 256
        wq = W[o0 : o0 + 256, :] * scale
        wk = W[C + o0 : C + o0 + 256, :]
        wv = W[2 * C + o0 : 2 * C + o0 + 256, :]
        wt = np.ascontiguousarray(
            np.concatenate([wq.T, wk.T, wv.T], axis=1).astype(bf)
        )
        bq = b[o0 : o0 + 256] * scale
        bk = b[C + o0 : C + o0 + 256]
        bvv = np.ascontiguousarray(b[2 * C + o0 : 2 * C + o0 + 256], dtype=np.float32)
        bcol = np.ascontiguousarray(np.concatenate([bq, bk]), dtype=np.float32)
        in_maps.append({"xt": xts[bb], "wt": wt, "bcol": bcol, "bv": bvv})
    return in_maps


def kernel(x: np.ndarray, W: np.ndarray, b: np.ndarray) -> np.ndarray:
    from concourse.bass_utils import run_bass_kernel_spmd

    x = np.asarray(x, dtype=np.float32)
    W = np.asarray(W, dtype=np.float32)
    b = np.asarray(b, dtype=np.float32)

    nc = _get_nc()
    in_maps = make_in_maps(x, W, b)
    res = run_bass_kernel_spmd(nc, in_maps, core_ids=list(range(NCORES)))

    y = np.empty((B, T, C), dtype=np.float32)
    for core in range(NCORES):
        bb, g = core // 4, core % 4
        y[bb, :, g * 256 : (g + 1) * 256] = (
            res.results[core]["out"].astype(np.float32).T
        )
    return y
